# revision 12
# baseline (speedup 1.0000x reference)
"""Causal self-attention (B=2, S=4096, D=512, H=8) on 8 Trainium2 NeuronCores.

Sharding: tensor-parallel over heads. Core h computes head h for both batch
elements: QKV projections for its head, causal flash attention, and its
partial (unnormalized) o_proj contribution y_h = U_h @ Wo[h*64:(h+1)*64, :]
plus the per-query softmax denominators L_h. The host computes
sum_h(y_h / L_h) + bo.

Everything on-chip is bf16 except psum (f32) and the o_proj (f32r).
HW matmul cost ~= moving free size in cycles regardless of dtype, so the
layout minimizes total streamed rows:
  - xT bf16 [512, 8192] streams in as [128f, 4c, 512t] tiles. Q/K projection:
    one [128, 512] psum ([Q.T; K.T]); V is computed NATURALLY (tokens on
    partitions): V[128t, 64] = xt_tile.T @ Wv chunk, 64-row streams, which
    also kills the V transposes. V' = [V | ones] bf16 (row sums -> L).
  - QT2 [128, 4096]/batch: Q.T dup'd into both partition halves (SBUF DMA).
    KTZ [128, 4096]/batch: zeros in partitions 0:64 (one-time DVE memset),
    K.T in 64:128, so S.T matmuls run K=128.
  - S.T chunk = 2 k-tiles -> psum [128, 2, 512]; one Scalar exp per chunk
    PSUM->SBUF produces P.T bf16; diagonal chunks get a 0/1 causal mask
    multiply on DVE.
  - AV: U'[65, 512] += V'_kt.T @ P.T_kt per k-tile; row 64 accumulates L.
    AV for chunk j is emitted after the S.T pair of chunk j+1.
  - y tiles [128q, 512] = U.T @ Wo_h (f32r, K=64), written out as bf16
    (unnormalized) with f32 L.
"""

import sys

for _p in ("/opt/trn_rl_repo", "/root/.axon_site/_ro/trn_rl_repo"):
    if _p not in sys.path:
        sys.path.insert(0, _p)

import numpy as np

import concourse.bass as bass
import concourse.mybir as mybir
import concourse.tile as tile
from concourse import bacc
from concourse.bass_utils import run_bass_kernel_spmd

B = 2
S = 4096
D = 512
H = 8
HD = 64
TOK = B * S          # 8192
NKT = S // 128       # 32 k-tiles per batch
NQB = S // 512       # 8 q-blocks per batch
SCALE = HD ** -0.5

F32 = mybir.dt.float32
F32R = mybir.dt.float32r
BF16 = mybir.dt.bfloat16

_CACHE = {}


def _build():
    nc = bacc.Bacc("TRN2", target_bir_lowering=False, debug=False, num_devices=8)

    xt_d = nc.dram_tensor("xt", [D, TOK], BF16, kind="ExternalInput")
    wqk_d = nc.dram_tensor("wqk", [D, 128], BF16, kind="ExternalInput")
    wv_d = nc.dram_tensor("wv", [D, HD], BF16, kind="ExternalInput")
    wo_d = nc.dram_tensor("wo", [128, D], F32R, kind="ExternalInput")
    bqk_d = nc.dram_tensor("bqk", [128, 1], F32, kind="ExternalInput")
    bvrow_d = nc.dram_tensor("bvrow", [128, 4, HD], BF16, kind="ExternalInput")
    mask_d = nc.dram_tensor("mask", [128, 4, 512], BF16, kind="ExternalInput")
    y_d = nc.dram_tensor("y", [TOK, D], BF16, kind="ExternalOutput")
    l_d = nc.dram_tensor("l", [TOK], F32R, kind="ExternalOutput")

    xt_r = xt_d.ap().rearrange("(c p) t -> p c t", p=128)      # [128, 4, 8192]
    wqk_r = wqk_d.ap().rearrange("(c p) m -> p c m", p=128)    # [128, 4, 128]
    wv_r = wv_d.ap().rearrange("(c p) m -> p c m", p=128)      # [128, 4, 64]

    with tile.TileContext(nc) as tc:
        import contextlib

        with contextlib.ExitStack() as ctx:
            singles = ctx.enter_context(tc.tile_pool(name="singles", bufs=1))
            xpool = ctx.enter_context(tc.tile_pool(name="xt", bufs=3))
            ptpool = ctx.enter_context(tc.tile_pool(name="pt", bufs=4))
            upool = ctx.enter_context(tc.tile_pool(name="usb", bufs=2))
            ypool = ctx.enter_context(tc.tile_pool(name="ysb", bufs=4))

            ps_st = ctx.enter_context(
                tc.tile_pool(name="ps_st", bufs=2, space="PSUM")
            )
            ps_u = ctx.enter_context(tc.tile_pool(name="ps_u", bufs=2, space="PSUM"))
            ps_misc = ctx.enter_context(
                tc.tile_pool(name="ps_misc", bufs=2, space="PSUM")
            )

            # --- constants / weights -----------------------------------
            wqk_sb = singles.tile([128, 4, 128], BF16)
            wv_sb = singles.tile([128, 4, HD], BF16)
            wo_sb = singles.tile([128, D], F32R)
            bqk_sb = singles.tile([128, 1], F32)
            bvrow_sb = singles.tile([128, 4, HD], BF16)
            mask_sb = singles.tile([128, 4, 512], BF16)
            # wqk/bqk/wv/bvrow load first so proj(0,0) starts ASAP;
            # mask/wo are deferred until after proj(0,0) is emitted.
            nc.sync.dma_start(out=wqk_sb, in_=wqk_r)
            nc.sync.dma_start(out=bqk_sb, in_=bqk_d.ap())
            nc.sync.dma_start(out=wv_sb, in_=wv_r)
            nc.sync.dma_start(out=bvrow_sb, in_=bvrow_d.ap())

            # --- persistent per-batch activation buffers ---------------
            qt2 = [
                singles.tile([128, S], BF16, tag=f"qt2_{b}", name=f"qt2_{b}")
                for b in range(B)
            ]
            ktz = [
                singles.tile([128, S], BF16, tag=f"ktz_{b}", name=f"ktz_{b}")
                for b in range(B)
            ]
            vp = [
                singles.tile([128, NKT, 65], BF16, tag=f"vp_{b}", name=f"vp_{b}")
                for b in range(B)
            ]
            for b in range(B):
                nc.vector.memset(vp[b][:, :, 64:65], 1.0)
                # zero the top half of KTZ: makes S.T matmuls full K=128
                # (K=64 matmuls stream ~1.5x slower per row on HW)
                nc.vector.memset(ktz[b][0:64, :], 0)

            def proj_block(b, tb):
                """Projections for 512 tokens (block tb of batch b)."""
                t0 = b * S + tb * 512
                xt_sb = xpool.tile([128, 4, 512], BF16, tag="xt")
                if b == 0 and tb == 0:
                    # per-chunk loads so the first matmul starts ASAP
                    for c in range(4):
                        nc.sync.dma_start(
                            out=xt_sb[:, c, :], in_=xt_r[:, c, t0 : t0 + 512]
                        )
                else:
                    nc.sync.dma_start(out=xt_sb, in_=xt_r[:, :, t0 : t0 + 512])

                qk_ps = ps_misc.tile([128, 512], F32, tag="m")
                for c in range(4):
                    nc.tensor.matmul(
                        qk_ps,
                        wqk_sb[:, c, :],
                        xt_sb[:, c, :],
                        start=(c == 0),
                        stop=(c == 3),
                    )

                # Q.T -> QT2 top half (+bias), then DMA-dup to bottom half
                cols = slice(tb * 512, (tb + 1) * 512)
                nc.vector.tensor_scalar_add(
                    qt2[b][0:64, cols], qk_ps[0:64, :], bqk_sb[0:64, 0:1]
                )
                nc.sync.dma_start(out=qt2[b][64:128, cols], in_=qt2[b][0:64, cols])

                # K.T -> KTZ bottom half (+bias); top half is zeros
                nc.vector.tensor_scalar_add(
                    ktz[b][64:128, cols], qk_ps[64:128, :], bqk_sb[64:128, 0:1]
                )

                # V natural per 128-token subtile: V[128t, 64] = xt.T @ Wv;
                # all four subtiles accumulate in one psum tile, one DVE add.
                v_ps = ps_misc.tile([128, 4, HD], F32, tag="m")
                for j in range(4):
                    for c in range(4):
                        nc.tensor.matmul(
                            v_ps[:, j, :],
                            xt_sb[:, c, j * 128 : (j + 1) * 128],
                            wv_sb[:, c, :],
                            start=(c == 0),
                            stop=(c == 3),
                            skip_group_check=True,
                        )
                nc.vector.tensor_add(
                    vp[b][:, 4 * tb : 4 * tb + 4, 0:64], v_ps, bvrow_sb
                )

            def attn_core(b, qb):
                """S.T + exp + mask + AV for q-block qb; leaves U' in u_sb."""
                q0 = qb * 512
                u_ps = ps_u.tile([65, 512], F32, tag="u")
                n_chunks = 2 * (qb + 1)  # chunks of 2 k-tiles

                def emit_av(pt, j):
                    # last chunk is computed only for queries 256:512
                    qs = slice(256, 512) if j == n_chunks - 1 else slice(0, 512)
                    for j2 in range(2):
                        kt = 2 * j + j2
                        nc.tensor.matmul(
                            u_ps[:, qs],
                            vp[b][:, kt, :],
                            pt[:, j2, :],
                            start=(kt == 0),
                            stop=(kt == 2 * n_chunks - 1),
                            skip_group_check=True,
                        )

                pts = []
                for j in range(n_chunks):
                    # Last chunk: keys q0+256..q0+512 only can see them, so
                    # S.T / exp / mask / AV all run at half width.
                    w = 256 if j == n_chunks - 1 else 512
                    qlo = q0 + 512 - w
                    stf = ps_st.tile([128, 2, 512], F32, tag="st")
                    st = stf[:, :, 0:w]
                    for j2 in range(2):
                        kt = 2 * j + j2
                        nc.tensor.matmul(
                            st[:, j2, :],
                            ktz[b][:, kt * 128 : (kt + 1) * 128],
                            qt2[b][:, qlo : qlo + w],
                            start=True,
                            stop=True,
                        )
                    ptf = ptpool.tile([128, 2, 512], BF16, tag="pt")
                    pt = ptf[:, :, 0:w]
                    nc.scalar.activation(
                        pt, st, mybir.ActivationFunctionType.Exp, scale=SCALE
                    )
                    if j >= n_chunks - 2:  # diagonal chunks: causal mask
                        d0 = (j % 2) * 2
                        nc.vector.tensor_mul(
                            pt, pt, mask_sb[:, d0 : d0 + 2, 512 - w : 512]
                        )
                    pts.append(pt)
                    if j >= 2:  # AV trails 2 chunks so exp never stalls the PE
                        emit_av(pts[j - 2], j - 2)
                emit_av(pts[n_chunks - 2], n_chunks - 2)
                # queries 0:256 are final before the (narrow) last chunk lands.
                # Rows 65:128 are a DMA dup of rows 0:63 (finite junk) so the
                # o_proj can run K=128 (K=64 matmuls stream ~2x slower); the
                # host zero-pads wo rows 64:128 to cancel them.
                u_sb = upool.tile([128, 512], F32R, tag="u")
                nc.vector.tensor_copy(u_sb[0:65, 0:256], u_ps[:, 0:256])
                nc.sync.dma_start(
                    out=u_sb[65:128, 0:256], in_=u_sb[0:63, 0:256]
                )
                emit_av(pts[n_chunks - 1], n_chunks - 1)
                nc.vector.tensor_copy(u_sb[0:65, 256:512], u_ps[:, 256:512])
                nc.sync.dma_start(
                    out=u_sb[65:128, 256:512], in_=u_sb[0:63, 256:512]
                )

                row0 = b * S + q0
                nc.sync.dma_start(
                    out=l_d.ap()[row0 : row0 + 512].rearrange("(p c) -> p c", p=1),
                    in_=u_sb[64:65, :],
                )
                return u_sb, row0

            def attn_finish(u_sb, row0):
                """y = U.T @ Wo_h (unnormalized, K=64), bf16 out."""
                for j2 in range(4):
                    y_ps = ps_misc.tile([128, 512], F32, tag="m")
                    nc.tensor.matmul(
                        y_ps,
                        u_sb[:, j2 * 128 : (j2 + 1) * 128],
                        wo_sb,
                        start=True,
                        stop=True,
                    )
                    y_sb = ypool.tile([128, 512], BF16, tag="y")
                    nc.vector.tensor_copy(y_sb, y_ps)
                    r0 = row0 + j2 * 128
                    nc.sync.dma_start(out=y_d.ap()[r0 : r0 + 128, :], in_=y_sb)

            # Pipeline: proj(tb) enables attn(qb=tb); the o_proj of the
            # previous q-block fills the PE while DVE writes qt/kt/u.
            pending = None
            for idx in range(B * NQB):
                b, tb = divmod(idx, NQB)
                proj_block(b, tb)
                if idx == 0:  # deferred weight loads overlap proj(0,0)
                    nc.sync.dma_start(out=mask_sb, in_=mask_d.ap())
                    nc.sync.dma_start(out=wo_sb, in_=wo_d.ap())
                if pending is not None:
                    attn_finish(*pending)
                pending = attn_core(b, tb)
            attn_finish(*pending)

    nc.compile()
    return nc


def _prep_inputs(x, Wq, bq, Wk, bk, Wv, bv, Wo, bo):
    import ml_dtypes

    xt = np.ascontiguousarray(x.reshape(TOK, D).T).astype(ml_dtypes.bfloat16)
    mask = np.zeros((128, 4, 512), dtype=np.float32)
    p = np.arange(128)[:, None]
    c = np.arange(512)[None, :]
    for dd in range(4):
        mask[:, dd, :] = (p + 128 * dd <= c).astype(np.float32)
    mask = mask.astype(ml_dtypes.bfloat16)

    in_maps = []
    for h in range(H):
        hs = slice(h * HD, (h + 1) * HD)
        in_maps.append(
            {
                "xt": xt,
                "wqk": np.ascontiguousarray(
                    np.concatenate([Wq[:, hs], Wk[:, hs]], axis=1)
                ).astype(ml_dtypes.bfloat16),
                "wv": np.ascontiguousarray(Wv[:, hs]).astype(ml_dtypes.bfloat16),
                "wo": np.concatenate([Wo[hs, :], np.zeros((HD, D))], axis=0).astype(np.float32),
                "bqk": np.concatenate([bq[hs], bk[hs]]).reshape(128, 1).astype(
                    np.float32
                ),
                "bvrow": np.ascontiguousarray(
                    np.broadcast_to(bv[hs][None, None, :], (128, 4, HD))
                ).astype(ml_dtypes.bfloat16),
                "mask": mask,
            }
        )
    return in_maps


def _install_ntff_hook():
    """Register the axon NTFF profiling hook (test-only plumbing)."""
    import types

    try:
        from antenv.axon_hooks import set_axon_ntff_profile_hook  # noqa: F401
    except ImportError:
        m = types.ModuleType("antenv.axon_hooks")
        m._HOOK = None
        m.set_axon_ntff_profile_hook = lambda h: setattr(m, "_HOOK", h)
        m.get_axon_ntff_profile_hook = lambda: m._HOOK
        sys.modules["antenv.axon_hooks"] = m
        import antenv

        antenv.axon_hooks = m
    from antenv.axon_hooks import (
        get_axon_ntff_profile_hook,
        set_axon_ntff_profile_hook,
    )

    if get_axon_ntff_profile_hook() is None:
        import trn_agent_boot.trn_boot as tb

        set_axon_ntff_profile_hook(
            tb._ntff_profile_via_ctypes("/opt/axon/libaxon_pjrt.so")
        )


def kernel(x, Wq, bq, Wk, bk, Wv, bv, Wo, bo, _trace=False):
    x, Wq, bq, Wk, bk, Wv, bv, Wo, bo = (
        np.asarray(a, dtype=np.float32) for a in (x, Wq, bq, Wk, bk, Wv, bv, Wo, bo)
    )
    if "nc" not in _CACHE:
        _CACHE["nc"] = _build()
    nc = _CACHE["nc"]
    in_maps = _prep_inputs(x, Wq, bq, Wk, bk, Wv, bv, Wo, bo)
    kwargs = {}
    if _trace:
        _install_ntff_hook()
        kwargs = dict(trace=True, trace_cores=[0])
    res = run_bass_kernel_spmd(nc, in_maps, core_ids=list(range(8)), **kwargs)
    _CACHE["last_result"] = res
    y = np.zeros((TOK, D), dtype=np.float64)
    for r in res.results:
        y += r["y"].astype(np.float64) / r["l"].astype(np.float64)[:, None]
    y += bo[None, :]
    return y.astype(np.float32).reshape(B, S, D)


# revision 15
# speedup vs baseline: 1.0658x; 1.0658x over previous
"""Causal self-attention (B=2, S=4096, D=512, H=8) on 8 Trainium2 NeuronCores.

Sharding: tensor-parallel over heads. Core h computes head h for both batch
elements: QKV projections for its head, causal flash attention, and its
partial (unnormalized) o_proj contribution y_h = U_h @ Wo[h*64:(h+1)*64, :]
plus the per-query softmax denominators L_h. The host computes
sum_h(y_h / L_h) + bo.

Everything on-chip is bf16 except psum (f32) and the o_proj (f32r).
HW matmul cost ~= moving free size in cycles regardless of dtype, so the
layout minimizes total streamed rows:
  - xT bf16 [512, 8192] streams in as [128f, 4c, 512t] tiles. Q/K projection:
    one [128, 512] psum ([Q.T; K.T]); V is computed NATURALLY (tokens on
    partitions): V[128t, 64] = xt_tile.T @ Wv chunk, 64-row streams, which
    also kills the V transposes. V' = [V | ones] bf16 (row sums -> L).
  - QT2 [128, 4096]/batch: Q.T dup'd into both partition halves (SBUF DMA).
    KTZ [128, 4096]/batch: zeros in partitions 0:64 (one-time DVE memset),
    K.T in 64:128, so S.T matmuls run K=128.
  - S.T chunk = 2 k-tiles -> psum [128, 2, 512]; one Scalar exp per chunk
    PSUM->SBUF produces P.T bf16; diagonal chunks get a 0/1 causal mask
    multiply on DVE.
  - AV: U'[65, 512] += V'_kt.T @ P.T_kt per k-tile; row 64 accumulates L.
    AV for chunk j is emitted after the S.T pair of chunk j+1.
  - y tiles [128q, 512] = U.T @ Wo_h (f32r, K=64), written out as bf16
    (unnormalized) with f32 L.
"""

import sys

for _p in ("/opt/trn_rl_repo", "/root/.axon_site/_ro/trn_rl_repo"):
    if _p not in sys.path:
        sys.path.insert(0, _p)

import numpy as np

import concourse.bass as bass
import concourse.mybir as mybir
import concourse.tile as tile
from concourse import bacc
from concourse.bass_utils import run_bass_kernel_spmd

B = 2
S = 4096
D = 512
H = 8
HD = 64
TOK = B * S          # 8192
NKT = S // 128       # 32 k-tiles per batch
NQB = S // 512       # 8 q-blocks per batch
SCALE = HD ** -0.5

F32 = mybir.dt.float32
F32R = mybir.dt.float32r
BF16 = mybir.dt.bfloat16

_CACHE = {}


def _build():
    nc = bacc.Bacc("TRN2", target_bir_lowering=False, debug=False, num_devices=8)

    xt_d = nc.dram_tensor("xt", [D, TOK], BF16, kind="ExternalInput")
    wqk_d = nc.dram_tensor("wqk", [D, 128], BF16, kind="ExternalInput")
    wv_d = nc.dram_tensor("wv", [D, HD], BF16, kind="ExternalInput")
    wo_d = nc.dram_tensor("wo", [128, D], F32R, kind="ExternalInput")
    bqk_d = nc.dram_tensor("bqk", [128, 1], F32, kind="ExternalInput")
    bvrow_d = nc.dram_tensor("bvrow", [128, 4, HD], BF16, kind="ExternalInput")
    mask_d = nc.dram_tensor("mask", [128, 4, 512], BF16, kind="ExternalInput")
    y_d = nc.dram_tensor("y", [TOK, D], BF16, kind="ExternalOutput")
    l_d = nc.dram_tensor("l", [TOK], F32R, kind="ExternalOutput")

    xt_r = xt_d.ap().rearrange("(c p) t -> p c t", p=128)      # [128, 4, 8192]
    wqk_r = wqk_d.ap().rearrange("(c p) m -> p c m", p=128)    # [128, 4, 128]
    wv_r = wv_d.ap().rearrange("(c p) m -> p c m", p=128)      # [128, 4, 64]

    with tile.TileContext(nc) as tc:
        import contextlib

        with contextlib.ExitStack() as ctx:
            singles = ctx.enter_context(tc.tile_pool(name="singles", bufs=1))
            xpool = ctx.enter_context(tc.tile_pool(name="xt", bufs=3))
            ptpool = ctx.enter_context(tc.tile_pool(name="pt", bufs=4))
            upool = ctx.enter_context(tc.tile_pool(name="usb", bufs=2))
            ypool = ctx.enter_context(tc.tile_pool(name="ysb", bufs=4))

            ps_st = ctx.enter_context(
                tc.tile_pool(name="ps_st", bufs=2, space="PSUM")
            )
            ps_u = ctx.enter_context(tc.tile_pool(name="ps_u", bufs=2, space="PSUM"))
            ps_misc = ctx.enter_context(
                tc.tile_pool(name="ps_misc", bufs=2, space="PSUM")
            )

            # --- constants / weights -----------------------------------
            wqk_sb = singles.tile([128, 4, 128], BF16)
            wv_sb = singles.tile([128, 4, HD], BF16)
            wo_sb = singles.tile([128, D], F32R)
            bqk_sb = singles.tile([128, 1], F32)
            bvrow_sb = singles.tile([128, 4, HD], BF16)
            mask_sb = singles.tile([128, 4, 512], BF16)
            # wqk/bqk/wv/bvrow load first so proj(0,0) starts ASAP;
            # mask/wo are deferred until after proj(0,0) is emitted.
            nc.sync.dma_start(out=wqk_sb, in_=wqk_r)
            nc.sync.dma_start(out=bqk_sb, in_=bqk_d.ap())
            nc.sync.dma_start(out=wv_sb, in_=wv_r)
            nc.sync.dma_start(out=bvrow_sb, in_=bvrow_d.ap())

            # --- persistent per-batch activation buffers ---------------
            qt2 = [
                singles.tile([128, S], BF16, tag=f"qt2_{b}", name=f"qt2_{b}")
                for b in range(B)
            ]
            ktz = [
                singles.tile([128, S], BF16, tag=f"ktz_{b}", name=f"ktz_{b}")
                for b in range(B)
            ]
            vp = [
                singles.tile([128, NKT, 65], BF16, tag=f"vp_{b}", name=f"vp_{b}")
                for b in range(B)
            ]
            for b in range(B):
                nc.vector.memset(vp[b][:, :, 64:65], 1.0)
                # zero the top half of KTZ: makes S.T matmuls full K=128
                # (K=64 matmuls stream ~1.5x slower per row on HW)
                nc.vector.memset(ktz[b][0:64, :], 0)

            def proj_block(b, tb):
                """Projections for 512 tokens (block tb of batch b)."""
                t0 = b * S + tb * 512
                xt_sb = xpool.tile([128, 4, 512], BF16, tag="xt")
                if b == 0 and tb == 0:
                    # per-chunk loads so the first matmul starts ASAP
                    for c in range(4):
                        nc.sync.dma_start(
                            out=xt_sb[:, c, :], in_=xt_r[:, c, t0 : t0 + 512]
                        )
                else:
                    nc.sync.dma_start(out=xt_sb, in_=xt_r[:, :, t0 : t0 + 512])

                qk_ps = ps_misc.tile([128, 512], F32, tag="m")
                for c in range(4):
                    nc.tensor.matmul(
                        qk_ps,
                        wqk_sb[:, c, :],
                        xt_sb[:, c, :],
                        start=(c == 0),
                        stop=(c == 3),
                    )

                # Q.T -> QT2 top half (+bias), then DMA-dup to bottom half
                cols = slice(tb * 512, (tb + 1) * 512)
                nc.vector.tensor_scalar_add(
                    qt2[b][0:64, cols], qk_ps[0:64, :], bqk_sb[0:64, 0:1]
                )
                nc.sync.dma_start(out=qt2[b][64:128, cols], in_=qt2[b][0:64, cols])

                # K.T -> KTZ bottom half (+bias); top half is zeros
                nc.vector.tensor_scalar_add(
                    ktz[b][64:128, cols], qk_ps[64:128, :], bqk_sb[64:128, 0:1]
                )

                # V natural per 128-token subtile: V[128t, 64] = xt.T @ Wv;
                # all four subtiles accumulate in one psum tile, one DVE add.
                v_ps = ps_misc.tile([128, 4, HD], F32, tag="m")
                for j in range(4):
                    for c in range(4):
                        nc.tensor.matmul(
                            v_ps[:, j, :],
                            xt_sb[:, c, j * 128 : (j + 1) * 128],
                            wv_sb[:, c, :],
                            start=(c == 0),
                            stop=(c == 3),
                            skip_group_check=True,
                        )
                nc.vector.tensor_add(
                    vp[b][:, 4 * tb : 4 * tb + 4, 0:64], v_ps, bvrow_sb
                )

            def attn_core(b, qb):
                """S.T + exp + mask + AV for q-block qb; leaves U' in u_sb."""
                q0 = qb * 512
                u_ps = ps_u.tile([65, 512], F32, tag="u")
                n_chunks = 2 * (qb + 1)  # chunks of 2 k-tiles

                def emit_av(pt, j):
                    # last chunk is computed only for queries 256:512
                    qs = slice(256, 512) if j == n_chunks - 1 else slice(0, 512)
                    for j2 in range(2):
                        kt = 2 * j + j2
                        nc.tensor.matmul(
                            u_ps[:, qs],
                            vp[b][:, kt, :],
                            pt[:, j2, :],
                            start=(kt == 0),
                            stop=(kt == 2 * n_chunks - 1),
                            skip_group_check=True,
                        )

                pts = []
                for j in range(n_chunks):
                    # Last chunk: keys q0+256..q0+512 only can see them, so
                    # S.T / exp / mask / AV all run at half width.
                    w = 256 if j == n_chunks - 1 else 512
                    qlo = q0 + 512 - w
                    stf = ps_st.tile([128, 2, 512], F32, tag="st")
                    st = stf[:, :, 0:w]
                    for j2 in range(2):
                        kt = 2 * j + j2
                        nc.tensor.matmul(
                            st[:, j2, :],
                            ktz[b][:, kt * 128 : (kt + 1) * 128],
                            qt2[b][:, qlo : qlo + w],
                            start=True,
                            stop=True,
                        )
                    ptf = ptpool.tile([128, 2, 512], BF16, tag="pt")
                    pt = ptf[:, :, 0:w]
                    nc.scalar.activation(
                        pt, st, mybir.ActivationFunctionType.Exp, scale=SCALE
                    )
                    if j >= n_chunks - 2:  # diagonal chunks: causal mask
                        d0 = (j % 2) * 2
                        nc.vector.tensor_mul(
                            pt, pt, mask_sb[:, d0 : d0 + 2, 512 - w : 512]
                        )
                    pts.append(pt)
                    if j >= 2:  # AV trails 2 chunks so exp never stalls the PE
                        emit_av(pts[j - 2], j - 2)
                emit_av(pts[n_chunks - 2], n_chunks - 2)
                # queries 0:256 are final before the (narrow) last chunk lands.
                # Rows 65:128 are a DMA dup of rows 0:63 (finite junk) so the
                # o_proj can run K=128 (K=64 matmuls stream ~2x slower); the
                # host zero-pads wo rows 64:128 to cancel them.
                u_sb = upool.tile([128, 512], F32R, tag="u")
                nc.vector.memset(u_sb[64:128, :].bitcast(BF16), 0)
                nc.vector.tensor_copy(u_sb[0:65, 0:256], u_ps[:, 0:256])
                emit_av(pts[n_chunks - 1], n_chunks - 1)
                nc.vector.tensor_copy(u_sb[0:65, 256:512], u_ps[:, 256:512])

                row0 = b * S + q0
                nc.sync.dma_start(
                    out=l_d.ap()[row0 : row0 + 512].rearrange("(p c) -> p c", p=1),
                    in_=u_sb[64:65, :],
                )
                return u_sb, row0

            def attn_finish(u_sb, row0):
                """y = U.T @ Wo_h (unnormalized, K=64), bf16 out."""
                for j2 in range(4):
                    y_ps = ps_misc.tile([128, 512], F32, tag="m")
                    nc.tensor.matmul(
                        y_ps,
                        u_sb[:, j2 * 128 : (j2 + 1) * 128],
                        wo_sb,
                        start=True,
                        stop=True,
                    )
                    y_sb = ypool.tile([128, 512], BF16, tag="y")
                    nc.vector.tensor_copy(y_sb, y_ps)
                    r0 = row0 + j2 * 128
                    nc.sync.dma_start(out=y_d.ap()[r0 : r0 + 128, :], in_=y_sb)

            # Pipeline: proj(tb) enables attn(qb=tb); the o_proj of the
            # previous q-block fills the PE while DVE writes qt/kt/u.
            pending = None
            for idx in range(B * NQB):
                b, tb = divmod(idx, NQB)
                proj_block(b, tb)
                if idx == 0:  # deferred weight loads overlap proj(0,0)
                    nc.sync.dma_start(out=mask_sb, in_=mask_d.ap())
                    nc.sync.dma_start(out=wo_sb, in_=wo_d.ap())
                if pending is not None:
                    attn_finish(*pending)
                pending = attn_core(b, tb)
            attn_finish(*pending)

    nc.compile()
    return nc


def _prep_inputs(x, Wq, bq, Wk, bk, Wv, bv, Wo, bo):
    import ml_dtypes

    xt = np.ascontiguousarray(x.reshape(TOK, D).T).astype(ml_dtypes.bfloat16)
    mask = np.zeros((128, 4, 512), dtype=np.float32)
    p = np.arange(128)[:, None]
    c = np.arange(512)[None, :]
    for dd in range(4):
        mask[:, dd, :] = (p + 128 * dd <= c).astype(np.float32)
    mask = mask.astype(ml_dtypes.bfloat16)

    in_maps = []
    for h in range(H):
        hs = slice(h * HD, (h + 1) * HD)
        in_maps.append(
            {
                "xt": xt,
                "wqk": np.ascontiguousarray(
                    np.concatenate([Wq[:, hs], Wk[:, hs]], axis=1)
                ).astype(ml_dtypes.bfloat16),
                "wv": np.ascontiguousarray(Wv[:, hs]).astype(ml_dtypes.bfloat16),
                "wo": np.concatenate([Wo[hs, :], np.zeros((HD, D))], axis=0).astype(np.float32),
                "bqk": np.concatenate([bq[hs], bk[hs]]).reshape(128, 1).astype(
                    np.float32
                ),
                "bvrow": np.ascontiguousarray(
                    np.broadcast_to(bv[hs][None, None, :], (128, 4, HD))
                ).astype(ml_dtypes.bfloat16),
                "mask": mask,
            }
        )
    return in_maps


def _install_ntff_hook():
    """Register the axon NTFF profiling hook (test-only plumbing)."""
    import types

    try:
        from antenv.axon_hooks import set_axon_ntff_profile_hook  # noqa: F401
    except ImportError:
        m = types.ModuleType("antenv.axon_hooks")
        m._HOOK = None
        m.set_axon_ntff_profile_hook = lambda h: setattr(m, "_HOOK", h)
        m.get_axon_ntff_profile_hook = lambda: m._HOOK
        sys.modules["antenv.axon_hooks"] = m
        import antenv

        antenv.axon_hooks = m
    from antenv.axon_hooks import (
        get_axon_ntff_profile_hook,
        set_axon_ntff_profile_hook,
    )

    if get_axon_ntff_profile_hook() is None:
        import trn_agent_boot.trn_boot as tb

        set_axon_ntff_profile_hook(
            tb._ntff_profile_via_ctypes("/opt/axon/libaxon_pjrt.so")
        )


def kernel(x, Wq, bq, Wk, bk, Wv, bv, Wo, bo, _trace=False):
    x, Wq, bq, Wk, bk, Wv, bv, Wo, bo = (
        np.asarray(a, dtype=np.float32) for a in (x, Wq, bq, Wk, bk, Wv, bv, Wo, bo)
    )
    if "nc" not in _CACHE:
        _CACHE["nc"] = _build()
    nc = _CACHE["nc"]
    in_maps = _prep_inputs(x, Wq, bq, Wk, bk, Wv, bv, Wo, bo)
    kwargs = {}
    if _trace:
        _install_ntff_hook()
        kwargs = dict(trace=True, trace_cores=[0])
    res = run_bass_kernel_spmd(nc, in_maps, core_ids=list(range(8)), **kwargs)
    _CACHE["last_result"] = res
    y = np.zeros((TOK, D), dtype=np.float64)
    for r in res.results:
        y += r["y"].astype(np.float64) / r["l"].astype(np.float64)[:, None]
    y += bo[None, :]
    return y.astype(np.float32).reshape(B, S, D)


# revision 16
# speedup vs baseline: 1.2063x; 1.1318x over previous
"""Causal self-attention (B=2, S=4096, D=512, H=8) on 8 Trainium2 NeuronCores.

Sharding: tensor-parallel over heads. Core h computes head h for both batch
elements: QKV projections for its head, causal flash attention, and its
partial (unnormalized) o_proj contribution y_h = U_h @ Wo[h*64:(h+1)*64, :]
plus the per-query softmax denominators L_h. The host computes
sum_h(y_h / L_h) + bo.

Everything on-chip is bf16 except psum (f32) and the o_proj (f32r).
HW matmul cost ~= moving free size in cycles regardless of dtype, so the
layout minimizes total streamed rows:
  - xT bf16 [512, 8192] streams in as [128f, 4c, 512t] tiles. Q/K projection:
    one [128, 512] psum ([Q.T; K.T]); V is computed NATURALLY (tokens on
    partitions): V[128t, 64] = xt_tile.T @ Wv chunk, 64-row streams, which
    also kills the V transposes. V' = [V | ones] bf16 (row sums -> L).
  - QT2 [128, 4096]/batch: Q.T dup'd into both partition halves (SBUF DMA).
    KTZ [128, 4096]/batch: zeros in partitions 0:64 (one-time DVE memset),
    K.T in 64:128, so S.T matmuls run K=128.
  - S.T chunk = 2 k-tiles -> psum [128, 2, 512]; one Scalar exp per chunk
    PSUM->SBUF produces P.T bf16; diagonal chunks get a 0/1 causal mask
    multiply on DVE.
  - AV: U'[65, 512] += V'_kt.T @ P.T_kt per k-tile; row 64 accumulates L.
    AV for chunk j is emitted after the S.T pair of chunk j+1.
  - y tiles [128q, 512] = U.T @ Wo_h (f32r, K=64), written out as bf16
    (unnormalized) with f32 L.
"""

import sys

for _p in ("/opt/trn_rl_repo", "/root/.axon_site/_ro/trn_rl_repo"):
    if _p not in sys.path:
        sys.path.insert(0, _p)

import numpy as np

import concourse.bass as bass
import concourse.mybir as mybir
import concourse.tile as tile
from concourse import bacc
from concourse.bass_utils import run_bass_kernel_spmd

B = 2
S = 4096
D = 512
H = 8
HD = 64
TOK = B * S          # 8192
NKT = S // 128       # 32 k-tiles per batch
NQB = S // 512       # 8 q-blocks per batch
SCALE = HD ** -0.5

F32 = mybir.dt.float32
F32R = mybir.dt.float32r
BF16 = mybir.dt.bfloat16

_CACHE = {}


def _build():
    nc = bacc.Bacc("TRN2", target_bir_lowering=False, debug=False, num_devices=8)

    xt_d = nc.dram_tensor("xt", [D, TOK], BF16, kind="ExternalInput")
    wqk_d = nc.dram_tensor("wqk", [D, 128], BF16, kind="ExternalInput")
    wv_d = nc.dram_tensor("wv", [D, HD], BF16, kind="ExternalInput")
    wo_d = nc.dram_tensor("wo", [128, D], F32R, kind="ExternalInput")
    bqk_d = nc.dram_tensor("bqk", [128, 1], F32, kind="ExternalInput")
    bvrow_d = nc.dram_tensor("bvrow", [128, 4, HD], BF16, kind="ExternalInput")
    mask_d = nc.dram_tensor("mask", [128, 4, 512], BF16, kind="ExternalInput")
    y_d = nc.dram_tensor("y", [TOK, D], BF16, kind="ExternalOutput")
    l_d = nc.dram_tensor("l", [TOK], F32R, kind="ExternalOutput")

    xt_r = xt_d.ap().rearrange("(c p) t -> p c t", p=128)      # [128, 4, 8192]
    wqk_r = wqk_d.ap().rearrange("(c p) m -> p c m", p=128)    # [128, 4, 128]
    wv_r = wv_d.ap().rearrange("(c p) m -> p c m", p=128)      # [128, 4, 64]

    with tile.TileContext(nc) as tc:
        import contextlib

        with contextlib.ExitStack() as ctx:
            singles = ctx.enter_context(tc.tile_pool(name="singles", bufs=1))
            xpool = ctx.enter_context(tc.tile_pool(name="xt", bufs=3))
            ptpool = ctx.enter_context(tc.tile_pool(name="pt", bufs=4))
            upool = ctx.enter_context(tc.tile_pool(name="usb", bufs=3))
            ypool = ctx.enter_context(tc.tile_pool(name="ysb", bufs=4))

            ps_st = ctx.enter_context(
                tc.tile_pool(name="ps_st", bufs=2, space="PSUM")
            )
            ps_u = ctx.enter_context(tc.tile_pool(name="ps_u", bufs=2, space="PSUM"))
            ps_misc = ctx.enter_context(
                tc.tile_pool(name="ps_misc", bufs=2, space="PSUM")
            )

            # --- constants / weights -----------------------------------
            wqk_sb = singles.tile([128, 4, 128], BF16)
            wv_sb = singles.tile([128, 4, HD], BF16)
            wo_sb = singles.tile([128, D], F32R)
            bqk_sb = singles.tile([128, 1], F32)
            bvrow_sb = singles.tile([128, 4, HD], BF16)
            mask_sb = singles.tile([128, 4, 512], BF16)
            # wqk/bqk/wv/bvrow load first so proj(0,0) starts ASAP;
            # mask/wo are deferred until after proj(0,0) is emitted.
            nc.sync.dma_start(out=wqk_sb, in_=wqk_r)
            nc.sync.dma_start(out=bqk_sb, in_=bqk_d.ap())
            nc.sync.dma_start(out=wv_sb, in_=wv_r)
            nc.sync.dma_start(out=bvrow_sb, in_=bvrow_d.ap())

            # --- persistent per-batch activation buffers ---------------
            qt2 = [
                singles.tile([128, S], BF16, tag=f"qt2_{b}", name=f"qt2_{b}")
                for b in range(B)
            ]
            ktz = [
                singles.tile([128, S], BF16, tag=f"ktz_{b}", name=f"ktz_{b}")
                for b in range(B)
            ]
            vp = [
                singles.tile([128, NKT, 65], BF16, tag=f"vp_{b}", name=f"vp_{b}")
                for b in range(B)
            ]
            for b in range(B):
                nc.vector.memset(vp[b][:, :, 64:65], 1.0)
                # zero the top half of KTZ: makes S.T matmuls full K=128
                # (K=64 matmuls stream ~1.5x slower per row on HW)
                nc.vector.memset(ktz[b][0:64, :], 0)

            def proj_block(b, tb):
                """Projections for 512 tokens (block tb of batch b)."""
                t0 = b * S + tb * 512
                xt_sb = xpool.tile([128, 4, 512], BF16, tag="xt")
                if b == 0 and tb == 0:
                    # per-chunk loads so the first matmul starts ASAP
                    for c in range(4):
                        nc.sync.dma_start(
                            out=xt_sb[:, c, :], in_=xt_r[:, c, t0 : t0 + 512]
                        )
                else:
                    nc.sync.dma_start(out=xt_sb, in_=xt_r[:, :, t0 : t0 + 512])

                qk_ps = ps_misc.tile([128, 512], F32, tag="m")
                for c in range(4):
                    nc.tensor.matmul(
                        qk_ps,
                        wqk_sb[:, c, :],
                        xt_sb[:, c, :],
                        start=(c == 0),
                        stop=(c == 3),
                    )

                # Q.T -> QT2 top half (+bias), then DMA-dup to bottom half
                cols = slice(tb * 512, (tb + 1) * 512)
                nc.vector.tensor_scalar_add(
                    qt2[b][0:64, cols], qk_ps[0:64, :], bqk_sb[0:64, 0:1]
                )
                nc.sync.dma_start(out=qt2[b][64:128, cols], in_=qt2[b][0:64, cols])

                # K.T -> KTZ bottom half (+bias); top half is zeros
                nc.vector.tensor_scalar_add(
                    ktz[b][64:128, cols], qk_ps[64:128, :], bqk_sb[64:128, 0:1]
                )

                # V natural per 128-token subtile: V[128t, 64] = xt.T @ Wv;
                # all four subtiles accumulate in one psum tile, one DVE add.
                v_ps = ps_misc.tile([128, 4, HD], F32, tag="m")
                for j in range(4):
                    for c in range(4):
                        nc.tensor.matmul(
                            v_ps[:, j, :],
                            xt_sb[:, c, j * 128 : (j + 1) * 128],
                            wv_sb[:, c, :],
                            start=(c == 0),
                            stop=(c == 3),
                            skip_group_check=True,
                        )
                nc.vector.tensor_add(
                    vp[b][:, 4 * tb : 4 * tb + 4, 0:64], v_ps, bvrow_sb
                )

            def attn_core(b, qb):
                """S.T + exp + mask + AV for q-block qb; leaves U' in u_sb."""
                q0 = qb * 512
                u_ps = ps_u.tile([65, 512], F32, tag="u")
                n_chunks = 2 * (qb + 1)  # chunks of 2 k-tiles

                def emit_av(pt, j):
                    # last chunk is computed only for queries 256:512
                    qs = slice(256, 512) if j == n_chunks - 1 else slice(0, 512)
                    for j2 in range(2):
                        kt = 2 * j + j2
                        nc.tensor.matmul(
                            u_ps[:, qs],
                            vp[b][:, kt, :],
                            pt[:, j2, :],
                            start=(kt == 0),
                            stop=(kt == 2 * n_chunks - 1),
                            skip_group_check=True,
                        )

                pts = []
                for j in range(n_chunks):
                    # Last chunk: keys q0+256..q0+512 only can see them, so
                    # S.T / exp / mask / AV all run at half width.
                    w = 256 if j == n_chunks - 1 else 512
                    qlo = q0 + 512 - w
                    stf = ps_st.tile([128, 2, 512], F32, tag="st")
                    st = stf[:, :, 0:w]
                    for j2 in range(2):
                        kt = 2 * j + j2
                        nc.tensor.matmul(
                            st[:, j2, :],
                            ktz[b][:, kt * 128 : (kt + 1) * 128],
                            qt2[b][:, qlo : qlo + w],
                            start=True,
                            stop=True,
                        )
                    ptf = ptpool.tile([128, 2, 512], BF16, tag="pt")
                    pt = ptf[:, :, 0:w]
                    nc.scalar.activation(
                        pt, st, mybir.ActivationFunctionType.Exp, scale=SCALE
                    )
                    if j >= n_chunks - 2:  # diagonal chunks: causal mask
                        d0 = (j % 2) * 2
                        nc.vector.tensor_mul(
                            pt, pt, mask_sb[:, d0 : d0 + 2, 512 - w : 512]
                        )
                    pts.append(pt)
                    if j >= 2:  # AV trails 2 chunks so exp never stalls the PE
                        emit_av(pts[j - 2], j - 2)
                emit_av(pts[n_chunks - 2], n_chunks - 2)
                # queries 0:256 are final before the (narrow) last chunk lands.
                # Rows 65:128 are a DMA dup of rows 0:63 (finite junk) so the
                # o_proj can run K=128 (K=64 matmuls stream ~2x slower); the
                # host zero-pads wo rows 64:128 to cancel them.
                u_sb = upool.tile([128, 512], F32R, tag="u")
                nc.vector.memset(u_sb[64:128, :].bitcast(BF16), 0)
                nc.vector.tensor_copy(u_sb[0:65, 0:256], u_ps[:, 0:256])
                emit_av(pts[n_chunks - 1], n_chunks - 1)
                nc.vector.tensor_copy(u_sb[0:65, 256:512], u_ps[:, 256:512])

                row0 = b * S + q0
                nc.sync.dma_start(
                    out=l_d.ap()[row0 : row0 + 512].rearrange("(p c) -> p c", p=1),
                    in_=u_sb[64:65, :],
                )
                return u_sb, row0

            def attn_finish(u_sb, row0):
                """y = U.T @ Wo_h (unnormalized, K=64), bf16 out."""
                for j2 in range(4):
                    y_ps = ps_misc.tile([128, 512], F32, tag="m")
                    nc.tensor.matmul(
                        y_ps,
                        u_sb[:, j2 * 128 : (j2 + 1) * 128],
                        wo_sb,
                        start=True,
                        stop=True,
                    )
                    y_sb = ypool.tile([128, 512], BF16, tag="y")
                    nc.vector.tensor_copy(y_sb, y_ps)
                    r0 = row0 + j2 * 128
                    nc.sync.dma_start(out=y_d.ap()[r0 : r0 + 128, :], in_=y_sb)

            # Schedule: the attn work is triangular in qb, so pair batch-1
            # q-blocks (ascending) with batch-0 q-blocks (descending) for a
            # constant 18 exp-chunks per step; batch-0 projections (plus its
            # two smallest attns) form the prologue. o_proj of a finished
            # q-block fills the PE while DVE writes qt/kt/u of the next.
            pending = []
            for tb in range(NQB):
                proj_block(0, tb)
                if tb == 0:  # deferred weight loads overlap proj(0,0)
                    nc.sync.dma_start(out=mask_sb, in_=mask_d.ap())
                    nc.sync.dma_start(out=wo_sb, in_=wo_d.ap())
                if tb == 2:
                    pending.append(attn_core(0, 0))
                if tb == 5:
                    pending.append(attn_core(0, 1))
            for i in range(NQB):
                proj_block(1, i)
                if pending:
                    attn_finish(*pending.pop(0))
                pending.append(attn_core(1, i))
                if 7 - i >= 2:
                    if pending:
                        attn_finish(*pending.pop(0))
                    pending.append(attn_core(0, 7 - i))
            for p in pending:
                attn_finish(*p)

    nc.compile()
    return nc


def _prep_inputs(x, Wq, bq, Wk, bk, Wv, bv, Wo, bo):
    import ml_dtypes

    xt = np.ascontiguousarray(x.reshape(TOK, D).T).astype(ml_dtypes.bfloat16)
    mask = np.zeros((128, 4, 512), dtype=np.float32)
    p = np.arange(128)[:, None]
    c = np.arange(512)[None, :]
    for dd in range(4):
        mask[:, dd, :] = (p + 128 * dd <= c).astype(np.float32)
    mask = mask.astype(ml_dtypes.bfloat16)

    in_maps = []
    for h in range(H):
        hs = slice(h * HD, (h + 1) * HD)
        in_maps.append(
            {
                "xt": xt,
                "wqk": np.ascontiguousarray(
                    np.concatenate([Wq[:, hs], Wk[:, hs]], axis=1)
                ).astype(ml_dtypes.bfloat16),
                "wv": np.ascontiguousarray(Wv[:, hs]).astype(ml_dtypes.bfloat16),
                "wo": np.concatenate([Wo[hs, :], np.zeros((HD, D))], axis=0).astype(np.float32),
                "bqk": np.concatenate([bq[hs], bk[hs]]).reshape(128, 1).astype(
                    np.float32
                ),
                "bvrow": np.ascontiguousarray(
                    np.broadcast_to(bv[hs][None, None, :], (128, 4, HD))
                ).astype(ml_dtypes.bfloat16),
                "mask": mask,
            }
        )
    return in_maps


def _install_ntff_hook():
    """Register the axon NTFF profiling hook (test-only plumbing)."""
    import types

    try:
        from antenv.axon_hooks import set_axon_ntff_profile_hook  # noqa: F401
    except ImportError:
        m = types.ModuleType("antenv.axon_hooks")
        m._HOOK = None
        m.set_axon_ntff_profile_hook = lambda h: setattr(m, "_HOOK", h)
        m.get_axon_ntff_profile_hook = lambda: m._HOOK
        sys.modules["antenv.axon_hooks"] = m
        import antenv

        antenv.axon_hooks = m
    from antenv.axon_hooks import (
        get_axon_ntff_profile_hook,
        set_axon_ntff_profile_hook,
    )

    if get_axon_ntff_profile_hook() is None:
        import trn_agent_boot.trn_boot as tb

        set_axon_ntff_profile_hook(
            tb._ntff_profile_via_ctypes("/opt/axon/libaxon_pjrt.so")
        )


def kernel(x, Wq, bq, Wk, bk, Wv, bv, Wo, bo, _trace=False):
    x, Wq, bq, Wk, bk, Wv, bv, Wo, bo = (
        np.asarray(a, dtype=np.float32) for a in (x, Wq, bq, Wk, bk, Wv, bv, Wo, bo)
    )
    if "nc" not in _CACHE:
        _CACHE["nc"] = _build()
    nc = _CACHE["nc"]
    in_maps = _prep_inputs(x, Wq, bq, Wk, bk, Wv, bv, Wo, bo)
    kwargs = {}
    if _trace:
        _install_ntff_hook()
        kwargs = dict(trace=True, trace_cores=[0])
    res = run_bass_kernel_spmd(nc, in_maps, core_ids=list(range(8)), **kwargs)
    _CACHE["last_result"] = res
    y = np.zeros((TOK, D), dtype=np.float64)
    for r in res.results:
        y += r["y"].astype(np.float64) / r["l"].astype(np.float64)[:, None]
    y += bo[None, :]
    return y.astype(np.float32).reshape(B, S, D)
